# revision 1
# baseline (speedup 1.0000x reference)
"""Block-circulant process as a dense matmul on 8 Trainium2 NeuronCores.

The reference computes, per (out-block i, in-block j), a circular
cross-correlation of x's block j with c_ij = irfft(truncated W[i,j]).
That whole operation is linear in x:  out = x @ M  where M is a real
4096x4096 matrix of 32x32 circulant blocks:

    M[j*128 + v, i*128 + u] = c[i, j, (v - u) mod 128]

M depends only on the weights, so it is precomputed on the host; the
device work is a dense (4096x4096) @ (4096x4096) matmul, sharded over
8 cores as 4 batch-quarters x 2 outfeat-halves. Each core computes
outT_shard = M_shard.T-contract against xT_shard in fp32r (full PE
rate at N=512, ~1.5e-4 scale-relative accuracy).
"""

import numpy as np

B_SIZE = 128
K_HALF = B_SIZE // 2 + 1  # 65
K_TRUNC = 48
BATCH = 4096
IN_F = 4096
OUT_F = 4096

N_CORES = 8
R_GRID = 4  # batch split
S_GRID = 2  # out-feature split
BQ = BATCH // R_GRID  # 1024 batch rows per core
OH = OUT_F // S_GRID  # 2048 out features per core
K_TILES = IN_F // 128  # 32 contraction tiles
M_TILES = OH // 128  # 16 out tiles per core
N_CHUNK = 512
B_CHUNKS = BQ // N_CHUNK  # 2 batch chunks per core

_CACHE = {}
LAST_RESULTS = None
TRACE = False


def _build_nc():
    import concourse.bacc as bacc
    import concourse.mybir as mybir
    import concourse.tile as tile

    F32R = mybir.dt.float32r

    nc = bacc.Bacc(None, target_bir_lowering=False)
    xT = nc.declare_dram_parameter("xT", [IN_F, BQ], F32R, isOutput=False)
    # weights pre-tiled on host: [mi, k, 128 (k-rows), 128 (m-cols)]
    m = nc.declare_dram_parameter("m", [M_TILES, K_TILES, 128, 128], F32R,
                                  isOutput=False)
    oT = nc.declare_dram_parameter("oT", [OH, BQ], mybir.dt.float32,
                                   isOutput=True)

    with tile.TileContext(nc) as tc:
        with (
            tc.tile_pool(name="xpool", bufs=1) as xpool,
            tc.tile_pool(name="wpool", bufs=12) as wpool,
            tc.tile_pool(name="opool", bufs=4) as opool,
            tc.tile_pool(name="psum", bufs=6, space="PSUM") as psum,
        ):
            x_tiles = []
            for k in range(K_TILES):
                xt = xpool.tile([128, BQ], F32R, name=f"x_{k}")
                nc.sync.dma_start(xt[:], xT[k * 128:(k + 1) * 128, :])
                x_tiles.append(xt)

            for mi in range(M_TILES):
                pss = [
                    psum.tile([128, N_CHUNK], mybir.dt.float32, name="ps")
                    for _ in range(B_CHUNKS)
                ]
                for k in range(K_TILES):
                    w = wpool.tile([128, 128], F32R, name="w")
                    nc.sync.dma_start(w[:], m[mi, k])
                    for b in range(B_CHUNKS):
                        nc.tensor.matmul(
                            pss[b][:],
                            w[:],
                            x_tiles[k][:, b * N_CHUNK:(b + 1) * N_CHUNK],
                            start=(k == 0),
                            stop=(k == K_TILES - 1),
                        )
                for b in range(B_CHUNKS):
                    ot = opool.tile([128, N_CHUNK], mybir.dt.float32, name="ot")
                    nc.vector.tensor_copy(ot[:], pss[b][:])
                    nc.scalar.dma_start(
                        oT[mi * 128:(mi + 1) * 128,
                           b * N_CHUNK:(b + 1) * N_CHUNK],
                        ot[:],
                    )
    nc.finalize()
    return nc


def _get_nc():
    if "nc" not in _CACHE:
        _CACHE["nc"] = _build_nc()
    return _CACHE["nc"]


def _weights_matrix(W_real, W_imag):
    """Time-domain dense matrix M with M[j*B+v, i*B+u] = c[i,j,(v-u)%B]."""
    mask = (np.arange(K_HALF) < K_TRUNC).astype(np.float64)
    half = (W_real.astype(np.float64) + 1j * W_imag.astype(np.float64)) * mask
    c = np.fft.irfft(half, n=B_SIZE, axis=-1)  # (k_out, k_in, B)
    idx = (np.arange(B_SIZE)[:, None] - np.arange(B_SIZE)[None, :]) % B_SIZE
    blocks = c[:, :, idx]  # (i, j, v, u)
    M = blocks.transpose(1, 2, 0, 3).reshape(IN_F, OUT_F)  # rows (j,v), cols (i,u)
    return np.ascontiguousarray(M, dtype=np.float32)


def kernel(x, W_real, W_imag):
    global LAST_RESULTS
    from concourse.bass_utils import run_bass_kernel_spmd

    x = np.asarray(x, dtype=np.float32)
    M = _weights_matrix(np.asarray(W_real), np.asarray(W_imag))
    xt = np.ascontiguousarray(x.T)  # (IN_F, BATCH)

    in_maps = []
    for core in range(N_CORES):
        r, s = divmod(core, S_GRID)
        xT_shard = np.ascontiguousarray(xt[:, r * BQ:(r + 1) * BQ])
        Ms = M[:, s * OH:(s + 1) * OH]
        m_tiled = np.ascontiguousarray(
            Ms.reshape(K_TILES, 128, M_TILES, 128).transpose(2, 0, 1, 3)
        )
        in_maps.append({"xT": xT_shard, "m": m_tiled})

    nc = _get_nc()
    res = run_bass_kernel_spmd(nc, in_maps, list(range(N_CORES)), trace=TRACE)
    LAST_RESULTS = res

    out = np.empty((BATCH, OUT_F), np.float32)
    for core in range(N_CORES):
        r, s = divmod(core, S_GRID)
        out[r * BQ:(r + 1) * BQ, s * OH:(s + 1) * OH] = (
            res.results[core]["oT"].T
        )
    return out


# revision 3
# speedup vs baseline: 1.4389x; 1.4389x over previous
"""Block-circulant process as a dense matmul on 8 Trainium2 NeuronCores.

The reference computes, per (out-block i, in-block j), a circular
cross-correlation of x's block j with c_ij = irfft(truncated W[i,j]).
That whole operation is linear in x:  out = x @ M  where M is a real
4096x4096 matrix of 32x32 circulant blocks:

    M[j*128 + v, i*128 + u] = c[i, j, (v - u) mod 128]

M depends only on the weights, so it is precomputed on the host; the
device work is a dense (4096x4096) @ (4096x4096) matmul, sharded over
8 cores as 4 batch-quarters x 2 outfeat-halves. Each core computes
outT_shard = M_shard.T-contract against xT_shard in fp32r (full PE
rate at N=512, ~1.5e-4 scale-relative accuracy).
"""

import numpy as np

B_SIZE = 128
K_HALF = B_SIZE // 2 + 1  # 65
K_TRUNC = 48
BATCH = 4096
IN_F = 4096
OUT_F = 4096

N_CORES = 8
R_GRID = 4  # batch split
S_GRID = 2  # out-feature split
BQ = BATCH // R_GRID  # 1024 batch rows per core
OH = OUT_F // S_GRID  # 2048 out features per core
K_TILES = IN_F // 128  # 32 contraction tiles
M_TILES = OH // 128  # 16 out tiles per core
N_CHUNK = 512
B_CHUNKS = BQ // N_CHUNK  # 2 batch chunks per core

K_GROUPS = 4  # w-stream DMA granularity: 8 k-tiles per transfer
KG = K_TILES // K_GROUPS  # 8

_CACHE = {}
LAST_RESULTS = None
TRACE = False


def _build_nc():
    import concourse.bacc as bacc
    import concourse.mybir as mybir
    import concourse.tile as tile

    F32R = mybir.dt.float32r

    nc = bacc.Bacc(None, target_bir_lowering=False)
    xT = nc.declare_dram_parameter("xT", [IN_F, BQ], F32R, isOutput=False)
    # weights pre-tiled on host so each SBUF partition line is one
    # contiguous 4KB DRAM run: m[mi, g, p, kk*128 + c] = w-tile(k=g*KG+kk)[p, c]
    m = nc.declare_dram_parameter("m", [M_TILES, K_GROUPS, 128, KG * 128],
                                  F32R, isOutput=False)
    oT = nc.declare_dram_parameter("oT", [OH, BQ], mybir.dt.float32,
                                   isOutput=True)

    with tile.TileContext(nc) as tc:
        with (
            tc.tile_pool(name="xpool", bufs=1) as xpool,
            tc.tile_pool(name="wpool", bufs=8) as wpool,
            tc.tile_pool(name="opool", bufs=4) as opool,
            tc.tile_pool(name="psum", bufs=6, space="PSUM") as psum,
        ):
            x_tiles = []
            for k in range(K_TILES):
                xt = xpool.tile([128, BQ], F32R, name=f"x_{k}")
                nc.sync.dma_start(xt[:], xT[k * 128:(k + 1) * 128, :])
                x_tiles.append(xt)

            for mi in range(M_TILES):
                pss = [
                    psum.tile([128, N_CHUNK], mybir.dt.float32, name="ps")
                    for _ in range(B_CHUNKS)
                ]
                for g in range(K_GROUPS):
                    wm = wpool.tile([128, KG * 128], F32R, name="wm")
                    nc.sync.dma_start(wm[:], m[mi, g])
                    for kk in range(KG):
                        k = g * KG + kk
                        for b in range(B_CHUNKS):
                            nc.tensor.matmul(
                                pss[b][:],
                                wm[:, kk * 128:(kk + 1) * 128],
                                x_tiles[k][:, b * N_CHUNK:(b + 1) * N_CHUNK],
                                start=(k == 0),
                                stop=(k == K_TILES - 1),
                            )
                for b in range(B_CHUNKS):
                    ot = opool.tile([128, N_CHUNK], mybir.dt.float32, name="ot")
                    nc.vector.tensor_copy(ot[:], pss[b][:])
                    nc.scalar.dma_start(
                        oT[mi * 128:(mi + 1) * 128,
                           b * N_CHUNK:(b + 1) * N_CHUNK],
                        ot[:],
                    )
    nc.finalize()
    return nc


def _get_nc():
    if "nc" not in _CACHE:
        _CACHE["nc"] = _build_nc()
    return _CACHE["nc"]


def _weights_matrix(W_real, W_imag):
    """Time-domain dense matrix M with M[j*B+v, i*B+u] = c[i,j,(v-u)%B]."""
    mask = (np.arange(K_HALF) < K_TRUNC).astype(np.float64)
    half = (W_real.astype(np.float64) + 1j * W_imag.astype(np.float64)) * mask
    c = np.fft.irfft(half, n=B_SIZE, axis=-1)  # (k_out, k_in, B)
    idx = (np.arange(B_SIZE)[:, None] - np.arange(B_SIZE)[None, :]) % B_SIZE
    blocks = c[:, :, idx]  # (i, j, v, u)
    M = blocks.transpose(1, 2, 0, 3).reshape(IN_F, OUT_F)  # rows (j,v), cols (i,u)
    return np.ascontiguousarray(M, dtype=np.float32)


def kernel(x, W_real, W_imag):
    global LAST_RESULTS
    from concourse.bass_utils import run_bass_kernel_spmd

    x = np.asarray(x, dtype=np.float32)
    M = _weights_matrix(np.asarray(W_real), np.asarray(W_imag))
    xt = np.ascontiguousarray(x.T)  # (IN_F, BATCH)

    in_maps = []
    for core in range(N_CORES):
        r, s = divmod(core, S_GRID)
        xT_shard = np.ascontiguousarray(xt[:, r * BQ:(r + 1) * BQ])
        Ms = M[:, s * OH:(s + 1) * OH]
        # [k, p, mi, c] -> [mi, g, kk, p, c] -> [mi, g, p, kk, c]
        m_tiled = np.ascontiguousarray(
            Ms.reshape(K_GROUPS, KG, 128, M_TILES, 128)
            .transpose(3, 0, 2, 1, 4)
            .reshape(M_TILES, K_GROUPS, 128, KG * 128)
        )
        in_maps.append({"xT": xT_shard, "m": m_tiled})

    nc = _get_nc()
    res = run_bass_kernel_spmd(nc, in_maps, list(range(N_CORES)), trace=TRACE)
    LAST_RESULTS = res

    out = np.empty((BATCH, OUT_F), np.float32)
    for core in range(N_CORES):
        r, s = divmod(core, S_GRID)
        out[r * BQ:(r + 1) * BQ, s * OH:(s + 1) * OH] = (
            res.results[core]["oT"].T
        )
    return out


# revision 5
# speedup vs baseline: 1.4624x; 1.0163x over previous
"""Block-circulant process as a dense matmul on 8 Trainium2 NeuronCores.

The reference computes, per (out-block i, in-block j), a circular
cross-correlation of x's block j with c_ij = irfft(truncated W[i,j]).
That whole operation is linear in x:  out = x @ M  where M is a real
4096x4096 matrix of 32x32 circulant blocks:

    M[j*128 + v, i*128 + u] = c[i, j, (v - u) mod 128]

M depends only on the weights, so it is precomputed on the host; the
device work is a dense (4096x4096) @ (4096x4096) matmul, sharded over
8 cores as 4 batch-quarters x 2 outfeat-halves. Each core computes
outT_shard = M_shard.T-contract against xT_shard in fp32r (full PE
rate at N=512, ~1.5e-4 scale-relative accuracy).
"""

import numpy as np

B_SIZE = 128
K_HALF = B_SIZE // 2 + 1  # 65
K_TRUNC = 48
BATCH = 4096
IN_F = 4096
OUT_F = 4096

N_CORES = 8
R_GRID = 4  # batch split
S_GRID = 2  # out-feature split
BQ = BATCH // R_GRID  # 1024 batch rows per core
OH = OUT_F // S_GRID  # 2048 out features per core
K_TILES = IN_F // 128  # 32 contraction tiles
M_TILES = OH // 128  # 16 out tiles per core
N_CHUNK = 512
B_CHUNKS = BQ // N_CHUNK  # 2 batch chunks per core

K_GROUPS = 4  # w-stream DMA granularity: 8 k-tiles per transfer
KG = K_TILES // K_GROUPS  # 8

_CACHE = {}
LAST_RESULTS = None
TRACE = False


def _build_nc():
    import concourse.bacc as bacc
    import concourse.mybir as mybir
    import concourse.tile as tile

    F32R = mybir.dt.float32r

    nc = bacc.Bacc(None, target_bir_lowering=False)
    xT = nc.declare_dram_parameter("xT", [IN_F, BQ], F32R, isOutput=False)
    # weights pre-tiled on host so each SBUF partition line is one
    # contiguous 16KB DRAM run: m[mi, p, k*128 + c] = w-tile(k)[p, c]
    m = nc.declare_dram_parameter("m", [M_TILES, 128, K_TILES * 128],
                                  F32R, isOutput=False)
    oT = nc.declare_dram_parameter("oT", [OH, BQ], mybir.dt.float32,
                                   isOutput=True)

    with tile.TileContext(nc) as tc:
        with (
            tc.tile_pool(name="xpool", bufs=1) as xpool,
            tc.tile_pool(name="wpool", bufs=3) as wpool,
            tc.tile_pool(name="opool", bufs=4) as opool,
            tc.tile_pool(name="psum", bufs=6, space="PSUM") as psum,
        ):
            x_tiles = []
            for k in range(K_TILES):
                xt = xpool.tile([128, BQ], F32R, name=f"x_{k}")
                nc.sync.dma_start(xt[:], xT[k * 128:(k + 1) * 128, :])
                x_tiles.append(xt)

            for mi in range(M_TILES):
                pss = [
                    psum.tile([128, N_CHUNK], mybir.dt.float32, name="ps")
                    for _ in range(B_CHUNKS)
                ]
                wm = wpool.tile([128, K_TILES * 128], F32R, name="wm")
                nc.sync.dma_start(wm[:], m[mi])
                for k in range(K_TILES):
                    for b in range(B_CHUNKS):
                        nc.tensor.matmul(
                            pss[b][:],
                            wm[:, k * 128:(k + 1) * 128],
                            x_tiles[k][:, b * N_CHUNK:(b + 1) * N_CHUNK],
                            start=(k == 0),
                            stop=(k == K_TILES - 1),
                        )
                for b in range(B_CHUNKS):
                    ot = opool.tile([128, N_CHUNK], mybir.dt.float32, name="ot")
                    nc.vector.tensor_copy(ot[:], pss[b][:])
                    nc.scalar.dma_start(
                        oT[mi * 128:(mi + 1) * 128,
                           b * N_CHUNK:(b + 1) * N_CHUNK],
                        ot[:],
                    )
    nc.finalize()
    return nc


def _get_nc():
    if "nc" not in _CACHE:
        _CACHE["nc"] = _build_nc()
    return _CACHE["nc"]


def _weights_matrix(W_real, W_imag):
    """Time-domain dense matrix M with M[j*B+v, i*B+u] = c[i,j,(v-u)%B]."""
    mask = (np.arange(K_HALF) < K_TRUNC).astype(np.float64)
    half = (W_real.astype(np.float64) + 1j * W_imag.astype(np.float64)) * mask
    c = np.fft.irfft(half, n=B_SIZE, axis=-1)  # (k_out, k_in, B)
    idx = (np.arange(B_SIZE)[:, None] - np.arange(B_SIZE)[None, :]) % B_SIZE
    blocks = c[:, :, idx]  # (i, j, v, u)
    M = blocks.transpose(1, 2, 0, 3).reshape(IN_F, OUT_F)  # rows (j,v), cols (i,u)
    return np.ascontiguousarray(M, dtype=np.float32)


def kernel(x, W_real, W_imag):
    global LAST_RESULTS
    from concourse.bass_utils import run_bass_kernel_spmd

    x = np.asarray(x, dtype=np.float32)
    M = _weights_matrix(np.asarray(W_real), np.asarray(W_imag))
    xt = np.ascontiguousarray(x.T)  # (IN_F, BATCH)

    in_maps = []
    for core in range(N_CORES):
        r, s = divmod(core, S_GRID)
        xT_shard = np.ascontiguousarray(xt[:, r * BQ:(r + 1) * BQ])
        Ms = M[:, s * OH:(s + 1) * OH]
        # [k, p, mi, c] -> [mi, p, k, c] -> [mi, 128, K_TILES*128]
        m_tiled = np.ascontiguousarray(
            Ms.reshape(K_TILES, 128, M_TILES, 128)
            .transpose(2, 1, 0, 3)
            .reshape(M_TILES, 128, K_TILES * 128)
        )
        in_maps.append({"xT": xT_shard, "m": m_tiled})

    nc = _get_nc()
    res = run_bass_kernel_spmd(nc, in_maps, list(range(N_CORES)), trace=TRACE)
    LAST_RESULTS = res

    out = np.empty((BATCH, OUT_F), np.float32)
    for core in range(N_CORES):
        r, s = divmod(core, S_GRID)
        out[r * BQ:(r + 1) * BQ, s * OH:(s + 1) * OH] = (
            res.results[core]["oT"].T
        )
    return out


# revision 7
# speedup vs baseline: 1.6532x; 1.1305x over previous
"""Block-circulant process via frequency-domain factorization on 8 cores.

out = x @ M factorizes through the (truncated, 48-bin) real FFT:
  stage A: per in-block j:  S[(p,f), b] = sum_t F[t,(p,f)] xT[jB+t, b]
  stage M: per freq pair e: mid[(q,i), b] = sum_{p,j} W_e[(p,j),(q,i)] S
  stage C: per out-block i: out[t, b] = sum_{q,f} G[(q,f), t] mid

All stages are single K<=128 matmuls (no PSUM accumulation). The two
partition-regroups between stages bounce through DRAM with affine
scatter APs. Sharding: pure data-parallel over batch (x dim 0), all
weight operands replicated. fp32r throughout.

PE per core: 88 matmuls (~20us). HBM per core: ~41 MiB.
"""

import numpy as np

B = 128
K_HALF = B // 2 + 1  # 65
KT = 48  # frequency truncation
KI = 32
KO = 32
BATCH = 4096
IN_F = 4096
OUT_F = 4096

N_CORES = 8
BQ = BATCH // N_CORES  # 512 batch rows per core
NP = KT // 2  # 24 frequency pairs
FE = NP  # e index range

_CACHE = {}
LAST_RESULTS = None
TRACE = False


def _build_nc():
    import concourse.bacc as bacc
    import concourse.mybir as mybir
    import concourse.tile as tile

    F32R = mybir.dt.float32r
    F32 = mybir.dt.float32

    nc = bacc.Bacc(None, target_bir_lowering=False)
    xT = nc.declare_dram_parameter("xT", [IN_F, BQ], F32R, isOutput=False)
    fmat = nc.declare_dram_parameter("fmat", [128, 96], F32R, isOutput=False)
    gmat = nc.declare_dram_parameter("gmat", [96, 128], F32R, isOutput=False)
    wmid = nc.declare_dram_parameter("wmid", [128, NP * 128], F32R,
                                     isOutput=False)
    oT = nc.declare_dram_parameter("oT", [OUT_F, BQ], F32, isOutput=True)

    # DRAM intermediates, laid out so stages M and C each load their whole
    # input with ONE contiguous DMA (48/64KB partition lines)
    # sS[fl*64 + p*32 + j, e*BQ + b]
    sS = nc.dram_tensor("sS", [128, NP * BQ], F32R)
    # cmid[q*48 + f, i*BQ + b]
    cmid = nc.dram_tensor("cmid", [96, KO * BQ], F32R)

    # views for the scattered writes
    sS_v = sS.rearrange("(fl p j) (e b) -> fl j p e b", fl=2, p=2, e=NP)
    cmid_v = cmid.rearrange("(q fe fl) (i b) -> fl fe q i b", fl=2, fe=FE,
                            i=KO)

    with tile.TileContext(nc) as tc:
        with (
            tc.tile_pool(name="cpool", bufs=1) as cpool,
            tc.tile_pool(name="xpool", bufs=8) as xpool,
            tc.tile_pool(name="spool", bufs=12) as spool,
            tc.tile_pool(name="bigpool", bufs=2) as bigpool,
            tc.tile_pool(name="opool", bufs=6) as opool,
            tc.tile_pool(name="psum", bufs=2, space="PSUM") as psum,
        ):
            f_t = cpool.tile([128, 96], F32R, name="f_t")
            nc.sync.dma_start(f_t[:], fmat[:])
            g_t = cpool.tile([96, 128], F32R, name="g_t")
            nc.sync.dma_start(g_t[:], gmat[:])
            # all 24 middle weight blocks in one DMA
            w_all = cpool.tile([128, NP * 128], F32R, name="w_all")
            nc.sync.dma_start(w_all[:], wmid[:])

            lanes = [nc.scalar, nc.sync, nc.gpsimd]

            # ---- stage A: 32 matmuls + scattered DRAM writes ----
            for j in range(KI):
                x_t = xpool.tile([128, BQ], F32R, name="x_t")
                (nc.sync if j % 2 == 0 else nc.scalar).dma_start(
                    x_t[:], xT[j * 128:(j + 1) * 128, :])
                ps = psum.tile([96, BQ], mybir.dt.float32, name="ps_a",
                               tag="ps_a")
                nc.tensor.matmul(ps[:], f_t[:], x_t[:], start=True, stop=True)
                s_t = spool.tile([96, BQ], F32, name="s_t")
                nc.vector.tensor_copy(s_t[:], ps[:])
                for fl in range(2):
                    nc.gpsimd.dma_start(
                        sS_v[fl, j],
                        s_t[fl * 48:(fl + 1) * 48, :].bitcast(F32R),
                    )

            # ---- stage M: grouped reads (6 pairs/DMA) + 24 matmuls ----
            EG = 6
            for g in range(NP // EG):
                m_g = bigpool.tile([128, EG * BQ], F32R, name="m_g",
                                   tag="big")
                nc.sync.dma_start(m_g[:], sS[:, g * EG * BQ:(g + 1) * EG * BQ])
                for ee in range(EG):
                    e = g * EG + ee
                    ps = psum.tile([128, BQ], mybir.dt.float32, name="ps_m",
                                   tag="ps_m")
                    nc.tensor.matmul(ps[:], w_all[:, e * 128:(e + 1) * 128],
                                     m_g[:, ee * BQ:(ee + 1) * BQ],
                                     start=True, stop=True)
                    m_out = opool.tile([128, BQ], F32, name="m_out", tag="mo")
                    nc.vector.tensor_copy(m_out[:], ps[:])
                    for fl in range(2):
                        (nc.scalar if fl == 0 else nc.gpsimd).dma_start(
                            cmid_v[fl, e],
                            m_out[fl * 64:(fl + 1) * 64, :].bitcast(F32R),
                        )

            # ---- stage C: grouped reads (8 i/DMA) + 32 matmuls ----
            IG = 8
            for g in range(KO // IG):
                c_g = bigpool.tile([96, IG * BQ], F32R, name="c_g", tag="big")
                nc.sync.dma_start(c_g[:],
                                  cmid[:, g * IG * BQ:(g + 1) * IG * BQ])
                for ii in range(IG):
                    i = g * IG + ii
                    ps = psum.tile([128, BQ], mybir.dt.float32, name="ps_c",
                                   tag="ps_c")
                    nc.tensor.matmul(ps[:], g_t[:],
                                     c_g[:, ii * BQ:(ii + 1) * BQ],
                                     start=True, stop=True)
                    o_t = opool.tile([128, BQ], F32, name="o_t")
                    nc.vector.tensor_copy(o_t[:], ps[:])
                    (nc.scalar if i % 2 == 0 else nc.gpsimd).dma_start(
                        oT[i * 128:(i + 1) * 128, :], o_t[:])
    nc.finalize()
    return nc


def _get_nc():
    if "nc" not in _CACHE:
        _CACHE["nc"] = _build_nc()
    return _CACHE["nc"]


def _host_weights(W_real, W_imag):
    """F [128,96], G [96,128], Wmid [24,128,128] (all float32)."""
    t = np.arange(B)[:, None].astype(np.float64)
    # F columns ordered (fl, p, e): f = 2e + fl; p=0 -> cos, p=1 -> -sin
    F = np.zeros((128, 96))
    for fl in range(2):
        for p in range(2):
            for e in range(FE):
                f = 2 * e + fl
                col = fl * 48 + p * 24 + e
                w = 2 * np.pi * f * t[:, 0] / B
                F[:, col] = np.cos(w) if p == 0 else -np.sin(w)
    # G rows ordered (q, f): q=0 -> scale*cos, q=1 -> -scale*sin
    G = np.zeros((96, 128))
    fs = np.arange(KT)
    scale = np.full(KT, 2.0 / B)
    scale[0] = 1.0 / B
    for q in range(2):
        for f in range(KT):
            w = 2 * np.pi * f * np.arange(B) / B
            G[q * 48 + f] = (scale[f] * np.cos(w) if q == 0
                             else -scale[f] * np.sin(w))
    # Wmid[e]: rows (fl, p, j), cols (fl, q, i); block-diag in fl
    Wr = W_real.astype(np.float64)
    Wi = W_imag.astype(np.float64)
    Wm = np.zeros((NP, 128, 128))
    for e in range(NP):
        for fl in range(2):
            f = 2 * e + fl
            r0, c0 = fl * 64, fl * 64
            # q=0: Re_out = Wr @ Re + Wi @ Im ; q=1: Im_out = Wr @ Im - Wi @ Re
            # rows (p=0: Re-in j), (p=1: Im-in j); cols (q, i)
            # lhsT[(p,j),(q,i)]: value multiplying S[p,j] into out[q,i]
            Wrf = Wr[:, :, f].T  # [j, i]
            Wif = Wi[:, :, f].T
            Wm[e, r0:r0 + 32, c0:c0 + 32] = Wrf          # p0 -> q0: Wr
            Wm[e, r0 + 32:r0 + 64, c0:c0 + 32] = Wif     # p1 -> q0: Wi
            Wm[e, r0:r0 + 32, c0 + 32:c0 + 64] = -Wif    # p0 -> q1: -Wi
            Wm[e, r0 + 32:r0 + 64, c0 + 32:c0 + 64] = Wrf  # p1 -> q1: Wr
    return (F.astype(np.float32), G.astype(np.float32), Wm.astype(np.float32))


def kernel(x, W_real, W_imag):
    global LAST_RESULTS
    from concourse.bass_utils import run_bass_kernel_spmd

    x = np.asarray(x, dtype=np.float32)
    F, G, Wm = _host_weights(np.asarray(W_real), np.asarray(W_imag))
    xt = np.ascontiguousarray(x.T)  # (IN_F, BATCH)

    in_maps = []
    for core in range(N_CORES):
        xT_shard = np.ascontiguousarray(xt[:, core * BQ:(core + 1) * BQ])
        wm_packed = np.ascontiguousarray(
            Wm.transpose(1, 0, 2).reshape(128, NP * 128))
        in_maps.append(
            {"xT": xT_shard, "fmat": F, "gmat": G, "wmid": wm_packed})

    nc = _get_nc()
    res = run_bass_kernel_spmd(nc, in_maps, list(range(N_CORES)), trace=TRACE)
    LAST_RESULTS = res

    out = np.empty((BATCH, OUT_F), np.float32)
    for core in range(N_CORES):
        out[core * BQ:(core + 1) * BQ, :] = res.results[core]["oT"].T
    return out
